# revision 14
# baseline (speedup 1.0000x reference)
"""Trainium2 Bass kernel for nn_Decoder_23089744183361.

Pointer-network-style GRU decoder: B=1024 sequences, S=256 steps, H=128.
Sharding: pure data parallel, batch split across 8 NeuronCores (128 rows
per core = exactly the SBUF partition width).

Per-core layout choices:
  - GRU hidden state lives TRANSPOSED in SBUF as hT [H=128p, B=128f] so it
    feeds matmuls (contraction over H on partitions) with no per-step
    transpose.
  - Gates are computed transposed ([gate_dim, B]); biases enter via K=1/K=2
    "bias matmuls" against indicator rows so PSUM accumulates gi+gh+b.
  - logits come out NATURAL ([B=128p, S=256f]) from one matmul with
    lhsT = hT, so argmax / masking / softmax are free-dim ops.
  - sigmoid(x) is computed as (1+tanh(x/2))/2 so the ONLY ACT functions
    used per step are {Tanh, Exp, Relu, Copy} which share one activation
    table set (no 1.3us table reloads); Ln runs once at the end.
  - argmax via DVE max/max_index (first-match ties, like jnp.argmax).
  - masking via an additive maskval tile (0 / -1e30) updated with a fused
    tensor_scalar(is_equal, mult) + min, on GPSIMD.
  - log_softmax: logp[chosen] = -log(sum(exp(masked - max))) since chosen
    IS the argmax -> no gather. exp+sum fused in one ACT op per step.
"""

import os
import sys
import functools

for _p in ("/opt/trn_rl_repo",):
    if _p not in sys.path and os.path.isdir(_p):
        sys.path.insert(0, _p)

import numpy as np

import concourse.bass as bass
import concourse.mybir as mybir
import concourse.tile as tile
from concourse import bacc
from concourse.bass_utils import run_bass_kernel_spmd

B, S, H = 1024, 256, 128
NCORES = 8
BL = B // NCORES  # 128 batch rows per core
H3 = 3 * H

F32 = mybir.dt.float32
U32 = mybir.dt.uint32
ALU = mybir.AluOpType
ACTF = mybir.ActivationFunctionType

NEG_BIG = -1.0e30  # stand-in for -inf in the mask (exp() underflows to 0)


def _mm(nc, out, lhsT, rhs, start, stop):
    nc.tensor.matmul(out, lhsT=lhsT, rhs=rhs, start=start, stop=stop)


def build_nc(n_steps=S, skip_masked=False, skip_topi=False, skip_gates=False):
    """Build the full unrolled Bass/Tile program (SPMD, same on all cores)."""
    nc = bacc.Bacc("TRN2", target_bir_lowering=False, debug=False,
                   num_devices=NCORES)

    # ---- DRAM I/O ----------------------------------------------------
    d_h0T = nc.dram_tensor("h0T", [H, BL], F32, kind="ExternalInput")
    d_WouT = nc.dram_tensor("WouT", [H, S], F32, kind="ExternalInput")
    d_WihT = nc.dram_tensor("WihT", [H, H3], F32, kind="ExternalInput")
    d_WhhT = nc.dram_tensor("WhhT", [H, H3], F32, kind="ExternalInput")
    d_WembT = nc.dram_tensor("WembT", [1, H], F32, kind="ExternalInput")
    d_bhn = nc.dram_tensor("bias_hn", [1, H], F32, kind="ExternalInput")
    d_bihn = nc.dram_tensor("b_ihn", [H, 1], F32, kind="ExternalInput")
    d_hbr = nc.dram_tensor("halfb_r", [H, 1], F32, kind="ExternalInput")
    d_hbz = nc.dram_tensor("halfb_z", [H, 1], F32, kind="ExternalInput")
    d_bout = nc.dram_tensor("b_out", [1, S], F32, kind="ExternalInput")
    d_ones1 = nc.dram_tensor("ones1", [1, H], F32, kind="ExternalInput")
    d_bemb = nc.dram_tensor("b_emb", [H, 1], F32, kind="ExternalInput")
    d_iota = nc.dram_tensor("iota", [BL, S], F32, kind="ExternalInput")
    d_ident = nc.dram_tensor("ident", [H, H], F32, kind="ExternalInput")

    d_tours = nc.dram_tensor("tours_o", [BL, S], F32, kind="ExternalOutput")
    d_logp = nc.dram_tensor("logp_o", [BL, S], F32, kind="ExternalOutput")

    with tile.TileContext(nc) as tc:
        import contextlib
        ctx = contextlib.ExitStack()
        with ctx:
            cpool = ctx.enter_context(tc.tile_pool(name="consts", bufs=1))
            spool = ctx.enter_context(tc.tile_pool(name="state", bufs=1))
            wpool = ctx.enter_context(tc.tile_pool(name="work", bufs=3))
            ppool = ctx.enter_context(
                tc.tile_pool(name="psum", bufs=1, space="PSUM"))

            # ---- constants into SBUF --------------------------------
            def cload(dram, shape, tag):
                t = cpool.tile(shape, F32, tag=tag)
                nc.sync.dma_start(t[:, :], dram[:, :])
                return t

            c_WouT = cload(d_WouT, [H, S], "WouT")
            c_WihT = cload(d_WihT, [H, H3], "WihT")
            c_WhhT = cload(d_WhhT, [H, H3], "WhhT")
            c_WembT = cload(d_WembT, [1, H], "WembT")
            c_bhn = cload(d_bhn, [1, H], "bhn")
            c_bihn = cload(d_bihn, [H, 1], "bihn")
            c_hbr = cload(d_hbr, [H, 1], "hbr")
            c_hbz = cload(d_hbz, [H, 1], "hbz")
            c_bout = cload(d_bout, [1, S], "bout")
            c_ones1 = cload(d_ones1, [1, H], "ones1")
            c_bemb = cload(d_bemb, [H, 1], "bemb")
            c_iota = cload(d_iota, [BL, S], "iota")
            c_ident = cload(d_ident, [H, H], "ident")

            # ---- persistent state -----------------------------------
            hT = spool.tile([H, BL], F32)
            embT = spool.tile([H, BL], F32)
            dec_row = spool.tile([1, BL], F32)
            maskval = spool.tile([BL, S], F32)       # 0 or NEG_BIG
            s_store = spool.tile([BL, S], F32)       # sum-exp per step
            chosen_st = spool.tile([BL, S], F32)     # chosen idx per step
            m8L = spool.tile([BL, 8], F32)           # topi top-8 values
            m8C = spool.tile([BL, 8], F32)           # chosen top-8 values
            negm1 = spool.tile([BL, 1], F32)         # -max(masked)
            topi8 = spool.tile([BL, 8], U32)
            chosen8 = spool.tile([BL, 8], U32)
            topi_f32 = spool.tile([BL, 1], F32)

            nc.sync.dma_start(hT[:, 0:BL], d_h0T[:, :])
            nc.gpsimd.memset(dec_row[:, 0:BL], 1.0)   # decoder_input = 1.0
            nc.gpsimd.memset(maskval[:, :], 0.0)
            nc.gpsimd.memset(s_store[:, :], 1.0)      # step0: logp = -ln(1)=0
            nc.gpsimd.memset(chosen_st[:, :], 0.0)    # step0: chosen = 0

            # ---- per-step PSUM tiles (one bank each) ----------------
            ps_r = ppool.tile([H, BL], F32)
            ps_z = ppool.tile([H, BL], F32)
            ps_in = ppool.tile([H, BL], F32)
            ps_hn = ppool.tile([H, BL], F32)
            ps_L = ppool.tile([BL, S], F32)     # logits (natural)
            ps_E = ppool.tile([H, BL], F32)     # emb outer product
            ps_T = ppool.tile([1, H], F32)      # transposed topi row

            for t in range(n_steps):
                last = (t == n_steps - 1)

                # ==== 1. embT = relu(W_emb (x) dec_row + b_emb) =======
                _mm(nc, ps_E[:, :], c_WembT[0:1, :], dec_row[0:1, :],
                    True, True)
                nc.scalar.activation(embT[:, :], ps_E[:, :],
                                     ACTF.Relu, bias=c_bemb[:, 0:1])

                # ==== 2. gate preactivations into PSUM ================
                _mm(nc, ps_r[:, :], c_WhhT[:, 0:H], hT[:, :], True, False)
                _mm(nc, ps_r[:, :], c_WihT[:, 0:H], embT[:, :], False, True)
                _mm(nc, ps_z[:, :], c_WhhT[:, H:2 * H], hT[:, :],
                    True, False)
                _mm(nc, ps_z[:, :], c_WihT[:, H:2 * H], embT[:, :],
                    False, True)
                _mm(nc, ps_in[:, :], c_WihT[:, 2 * H:H3], embT[:, :],
                    True, True)
                # ps_hn = 0.5*(gh_n + b_hn)   (0.5 folded into WhhT/bias)
                _mm(nc, ps_hn[:, :], c_bhn[0:1, :], c_ones1[0:1, :],
                    True, False)
                _mm(nc, ps_hn[:, :], c_WhhT[:, 2 * H:H3], hT[:, :],
                    False, True)

                # ==== 3. gates (sigmoid via tanh half-angle) ==========
                # tau = tanh(0.5*x + 0.5*b); sigmoid(x) = (1+tau)/2
                tau_r = wpool.tile([H, BL], F32, tag="tau_r")
                nc.scalar.activation(tau_r[:, :], ps_r[:, :], ACTF.Tanh,
                                     scale=0.5, bias=c_hbr[:, 0:1])
                tau_z = wpool.tile([H, BL], F32, tag="tau_z")
                nc.scalar.activation(tau_z[:, :], ps_z[:, :], ACTF.Tanh,
                                     scale=0.5, bias=c_hbz[:, 0:1])
                # tgate = (1+tau_r) * hn'   (= r*h_n with all the halves)
                tgate = wpool.tile([H, BL], F32, tag="tgate")
                nc.vector.scalar_tensor_tensor(
                    tgate[:, :], tau_r[:, :], 1.0, ps_hn[:, :],
                    ALU.add, ALU.mult)
                # v = i_n + tgate ; n = tanh(v + b_ihn)
                v_t = wpool.tile([H, BL], F32, tag="v")
                nc.vector.tensor_tensor(v_t[:, :], tgate[:, :],
                                        ps_in[:, :], ALU.add)
                n_t = wpool.tile([H, BL], F32, tag="n")
                nc.scalar.activation(n_t[:, :], v_t[:, :], ACTF.Tanh,
                                     bias=c_bihn[:, 0:1])
                # z = 0.5*tau_z + 0.5 ; h' = n + z*(h - n)
                z_sb = wpool.tile([H, BL], F32, tag="z_sb")
                nc.gpsimd.tensor_scalar(z_sb[:, :], tau_z[:, :],
                                        0.5, 0.5, ALU.mult, ALU.add)
                d_t = wpool.tile([H, BL], F32, tag="d_t")
                nc.vector.tensor_tensor(d_t[:, :], hT[:, 0:BL], n_t[:, :],
                                        ALU.subtract)
                e_t = wpool.tile([H, BL], F32, tag="e_t")
                nc.vector.tensor_tensor(e_t[:, :], z_sb[:, :], d_t[:, :],
                                        ALU.mult)
                nc.gpsimd.tensor_tensor(hT[:, 0:BL], n_t[:, :], e_t[:, :],
                                        ALU.add)

                # ==== 4. logits (natural layout) ======================
                _mm(nc, ps_L[:, :], c_ones1[0:1, :], c_bout[0:1, :],
                    True, False)
                _mm(nc, ps_L[:, :], hT[:, 0:BL], c_WouT[:, :], False, True)

                # ==== 5. topi -> dec_row (feeds step t+1) =============
                if not (last or skip_topi):
                    nc.vector.max(m8L[:, :], ps_L[:, :])
                    nc.vector.max_index(topi8[:, :], m8L[:, :], ps_L[:, :])
                    nc.gpsimd.tensor_copy(topi_f32[:, 0:1], topi8[:, 0:1])
                    nc.tensor.transpose(ps_T[0:1, :], topi_f32[:, 0:1],
                                        c_ident[:, :])
                    nc.scalar.activation(dec_row[0:1, 0:BL], ps_T[0:1, :],
                                         ACTF.Copy)

                # ==== 6. masked path (skipped at t=0) =================
                if t >= 1 and not skip_masked:
                    sel = wpool.tile([BL, S], F32, tag="sel")
                    nc.gpsimd.tensor_scalar(
                        sel[:, :], c_iota[:, :], chosen_st[:, t - 1:t],
                        NEG_BIG, ALU.is_equal, ALU.mult)
                    nc.gpsimd.tensor_tensor(maskval[:, :], maskval[:, :],
                                            sel[:, :], ALU.add)
                    masked = wpool.tile([BL, S], F32, tag="masked")
                    nc.vector.tensor_tensor(masked[:, :], ps_L[:, :],
                                            maskval[:, :], ALU.add)
                    nc.vector.max(m8C[:, :], masked[:, :])
                    nc.vector.max_index(chosen8[:, :], m8C[:, :],
                                        masked[:, :])
                    nc.gpsimd.tensor_copy(chosen_st[:, t:t + 1],
                                          chosen8[:, 0:1])
                    nc.gpsimd.tensor_scalar_mul(negm1[:, 0:1], m8C[:, 0:1],
                                                -1.0)
                    exps = wpool.tile([BL, S], F32, tag="exps")
                    nc.scalar.activation(exps[:, :], masked[:, :],
                                         ACTF.Exp, scale=1.0,
                                         bias=negm1[:, 0:1],
                                         accum_out=s_store[:, t:t + 1])

            # ---- finalize outputs -----------------------------------
            lnout = spool.tile([BL, S], F32)
            nc.scalar.activation(lnout[:, :], s_store[:, :], ACTF.Ln)
            nc.gpsimd.tensor_scalar_mul(lnout[:, :], lnout[:, :], -1.0)
            nc.sync.dma_start(d_tours[:, :], chosen_st[:, :])
            nc.sync.dma_start(d_logp[:, :], lnout[:, :])

    if not nc.is_finalized():
        nc.finalize()
    return nc


def _prep_consts(W_emb, b_emb, W_ih, W_hh, b_ih, b_hh, W_out, b_out):
    """Host-side constant packing (numpy, float32)."""
    WihT = np.ascontiguousarray(W_ih.T).astype(np.float32)      # [H, 3H]
    WhhT = np.ascontiguousarray(W_hh.T).astype(np.float32)      # [H, 3H]
    WhhT[:, 2 * H:] *= 0.5                                       # hn' fold
    bias_hn = (0.5 * b_hh[2 * H:]).reshape(1, H).astype(np.float32)
    b_ihn = b_ih[2 * H:].reshape(H, 1).astype(np.float32)
    halfb_r = (0.5 * (b_ih[0:H] + b_hh[0:H])).reshape(H, 1).astype(np.float32)
    halfb_z = (0.5 * (b_ih[H:2 * H] + b_hh[H:2 * H])).reshape(H, 1).astype(
        np.float32)
    consts = {
        "WouT": np.ascontiguousarray(W_out.T).astype(np.float32),
        "WihT": WihT,
        "WhhT": WhhT,
        "WembT": W_emb.T.reshape(1, H).astype(np.float32),
        "bias_hn": bias_hn,
        "b_ihn": b_ihn,
        "halfb_r": halfb_r,
        "halfb_z": halfb_z,
        "b_out": b_out.reshape(1, S).astype(np.float32),
        "ones1": np.ones((1, H), np.float32),
        "b_emb": b_emb.reshape(H, 1).astype(np.float32),
        "iota": np.broadcast_to(np.arange(S, dtype=np.float32),
                                (BL, S)).copy(),
        "ident": np.eye(H, dtype=np.float32),
    }
    return consts


@functools.lru_cache(maxsize=1)
def _get_nc():
    return build_nc(S)


def kernel(encoder_outputs, encoder_hidden, W_emb, b_emb, W_ih, W_hh,
           b_ih, b_hh, W_out, b_out, _nc=None, _trace=False, _tmpdir=None,
           **_unused):
    encoder_hidden = np.asarray(encoder_hidden, np.float32)
    consts = _prep_consts(
        np.asarray(W_emb, np.float32), np.asarray(b_emb, np.float32),
        np.asarray(W_ih, np.float32), np.asarray(W_hh, np.float32),
        np.asarray(b_ih, np.float32), np.asarray(b_hh, np.float32),
        np.asarray(W_out, np.float32), np.asarray(b_out, np.float32))

    h0 = encoder_hidden[0]  # [B, H]
    in_maps = []
    for c in range(NCORES):
        m = dict(consts)
        m["h0T"] = np.ascontiguousarray(h0[c * BL:(c + 1) * BL].T)
        in_maps.append(m)

    nc = _nc if _nc is not None else _get_nc()
    kw = {}
    if _trace:
        kw = dict(trace=True, tmpdir=_tmpdir)
    res = run_bass_kernel_spmd(nc, in_maps, list(range(NCORES)), **kw)
    kernel.last_results = res
    tours = np.concatenate([res.results[c]["tours_o"] for c in range(NCORES)],
                           axis=0)
    logp = np.concatenate([res.results[c]["logp_o"] for c in range(NCORES)],
                          axis=0)
    tours_out = tours[:, None, :].astype(np.float32)      # [B, 1, S]
    logp_out = logp[:, :, None].astype(np.float32)        # [B, S, 1]
    return tours_out, logp_out


if __name__ == "__main__":
    import time
    t0 = time.time()
    nc = build_nc(4)
    print(f"built 4-step nc in {time.time() - t0:.1f}s")


# revision 23
# speedup vs baseline: 44.8016x; 44.8016x over previous
"""Trainium2 Bass kernel for nn_Decoder_23089744183361.

Pointer-network-style GRU decoder: B=1024 sequences, S=256 steps, H=128.
Sharding: pure data parallel, batch split across 8 NeuronCores (128 rows
per core = exactly the SBUF partition width).

Per-core layout choices:
  - GRU hidden state lives TRANSPOSED in SBUF as hT [H=128p, B=128f] so it
    feeds matmuls (contraction over H on partitions) with no per-step
    transpose.
  - Gates are computed transposed ([gate_dim, B]); biases enter via K=1/K=2
    "bias matmuls" against indicator rows so PSUM accumulates gi+gh+b.
  - logits come out NATURAL ([B=128p, S=256f]) from one matmul with
    lhsT = hT, so argmax / masking / softmax are free-dim ops.
  - sigmoid(x) is computed as (1+tanh(x/2))/2 so the ONLY ACT functions
    used per step are {Tanh, Exp, Relu, Copy} which share one activation
    table set (no 1.3us table reloads); Ln runs once at the end.
  - argmax via DVE max/max_index (first-match ties, like jnp.argmax).
  - masking via an additive maskval tile (0 / -1e30) updated with a fused
    tensor_scalar(is_equal, mult) + min, on GPSIMD.
  - log_softmax: logp[chosen] = -log(sum(exp(masked - max))) since chosen
    IS the argmax -> no gather. exp+sum fused in one ACT op per step.
"""

import os
import sys
import functools

for _p in ("/opt/trn_rl_repo",):
    if _p not in sys.path and os.path.isdir(_p):
        sys.path.insert(0, _p)

import numpy as np

import concourse.bass as bass
import concourse.mybir as mybir
import concourse.tile as tile
from concourse import bacc
from concourse.bass_utils import run_bass_kernel_spmd

B, S, H = 1024, 256, 128
NCORES = 8
BL = B // NCORES  # 128 batch rows per core
H3 = 3 * H

F32 = mybir.dt.float32
U32 = mybir.dt.uint32
ALU = mybir.AluOpType
ACTF = mybir.ActivationFunctionType

NEG_BIG = -1.0e30  # stand-in for -inf in the mask (exp() underflows to 0)


def _mm(nc, out, lhsT, rhs, start, stop):
    nc.tensor.matmul(out, lhsT=lhsT, rhs=rhs, start=start, stop=stop)


def build_nc(n_steps=S, skip_masked=False, skip_topi=False, break_chain=False, reps=1):
    """Build the full unrolled Bass/Tile program (SPMD, same on all cores)."""
    nc = bacc.Bacc("TRN2", target_bir_lowering=False, debug=False,
                   num_devices=NCORES)

    # ---- DRAM I/O ----------------------------------------------------
    d_h0T = nc.dram_tensor("h0T", [H, BL], F32, kind="ExternalInput")
    d_WouT = nc.dram_tensor("WouT", [H, S], F32, kind="ExternalInput")
    d_WihT = nc.dram_tensor("WihT", [H, H3], F32, kind="ExternalInput")
    d_WhhT = nc.dram_tensor("WhhT", [H, H3], F32, kind="ExternalInput")
    d_WembT = nc.dram_tensor("WembT", [1, H], F32, kind="ExternalInput")
    d_bhn = nc.dram_tensor("bias_hn", [1, H], F32, kind="ExternalInput")
    d_bihn = nc.dram_tensor("b_ihn", [H, 1], F32, kind="ExternalInput")
    d_hbr = nc.dram_tensor("halfb_r", [H, 1], F32, kind="ExternalInput")
    d_hbz = nc.dram_tensor("halfb_z", [H, 1], F32, kind="ExternalInput")
    d_bout = nc.dram_tensor("b_out", [1, S], F32, kind="ExternalInput")
    d_ones1 = nc.dram_tensor("ones1", [1, H], F32, kind="ExternalInput")
    d_bemb = nc.dram_tensor("b_emb", [H, 1], F32, kind="ExternalInput")
    d_iota = nc.dram_tensor("iota", [BL, S], F32, kind="ExternalInput")
    d_ident = nc.dram_tensor("ident", [H, H], F32, kind="ExternalInput")

    d_tours = nc.dram_tensor("tours_o", [BL, S], F32, kind="ExternalOutput")
    d_logp = nc.dram_tensor("logp_o", [BL, S], F32, kind="ExternalOutput")

    with tile.TileContext(nc) as tc:
        import contextlib
        ctx = contextlib.ExitStack()
        with ctx:
            cpool = ctx.enter_context(tc.tile_pool(name="consts", bufs=1))
            spool = ctx.enter_context(tc.tile_pool(name="state", bufs=1))
            wpool = ctx.enter_context(tc.tile_pool(name="work", bufs=3))
            ppool = ctx.enter_context(
                tc.tile_pool(name="psum", bufs=1, space="PSUM"))

            # ---- constants into SBUF --------------------------------
            def cload(dram, shape, tag):
                t = cpool.tile(shape, F32, tag=tag)
                nc.sync.dma_start(t[:, :], dram[:, :])
                return t

            c_WouT = cload(d_WouT, [H, S], "WouT")
            c_WihT = cload(d_WihT, [H, H3], "WihT")
            c_WhhT = cload(d_WhhT, [H, H3], "WhhT")
            c_WembT = cload(d_WembT, [1, H], "WembT")
            c_bhn = cload(d_bhn, [1, H], "bhn")
            c_bihn = cload(d_bihn, [H, 1], "bihn")
            c_hbr = cload(d_hbr, [H, 1], "hbr")
            c_hbz = cload(d_hbz, [H, 1], "hbz")
            c_bout = cload(d_bout, [1, S], "bout")
            c_boutb = cpool.tile([BL, S], F32, tag="boutb")
            nc.sync.dma_start(c_boutb[:, :],
                              d_bout[:, :].broadcast_to((BL, S)))
            c_ones1 = cload(d_ones1, [1, H], "ones1")
            c_bemb = cload(d_bemb, [H, 1], "bemb")
            c_iota = cload(d_iota, [BL, S], "iota")
            c_ident = cload(d_ident, [H, H], "ident")
            c_WembB = cpool.tile([H, H], F32, tag="WembB")
            nc.sync.dma_start(c_WembB[:, :],
                              d_WembT[:, :].broadcast_to((H, H)))

            # ---- persistent state -----------------------------------
            rep_loop = tc.For_i(0, reps, 1) if reps > 1 else None
            if rep_loop is not None:
                rep_loop.__enter__()
            hT = spool.tile([H, BL], F32)
            embT = spool.tile([H, BL], F32)
            diag_t = spool.tile([H, BL], F32)
            maskval = spool.tile([BL, S], F32)       # 0 or NEG_BIG
            s_store = spool.tile([BL, S], F32)       # sum-exp per step
            chosen_st = spool.tile([BL, S], F32)     # chosen idx per step
            m8L = spool.tile([BL, 8], F32)           # topi top-8 values
            m8C = spool.tile([BL, 8], F32)           # chosen top-8 values
            negm1 = spool.tile([BL, 1], F32)         # -max(masked)
            topi8 = spool.tile([BL, 8], U32)
            chosen8 = spool.tile([BL, 8], U32)
            topi_f32 = spool.tile([BL, 1], F32)

            nc.sync.dma_start(hT[:, 0:BL], d_h0T[:, :])
            # step0 decoder_input = 1.0 -> diag = I
            nc.sync.dma_start(diag_t[:, :], d_ident[:, :])
            nc.gpsimd.memset(maskval[:, :], 0.0)
            nc.gpsimd.memset(s_store[:, :], 1.0)      # step0: logp = -ln(1)=0
            nc.gpsimd.memset(chosen_st[:, :], 0.0)    # step0: chosen = 0

            # ---- per-step PSUM tiles (one bank each) ----------------
            ps_r = ppool.tile([H, BL], F32)
            ps_z = ppool.tile([H, BL], F32)
            ps_in = ppool.tile([H, BL], F32)
            ps_hn = ppool.tile([H, BL], F32)
            ps_L0 = ppool.tile([BL, S], F32)    # logits (natural), even t
            ps_L1 = ppool.tile([BL, S], F32)    # logits (natural), odd t
            ps_E = ppool.tile([H, BL], F32)     # emb outer product

            for t in range(n_steps):
                last = (t == n_steps - 1)
                ps_L = ps_L0 if t % 2 == 0 else ps_L1
                ps_Lprev = ps_L1 if t % 2 == 0 else ps_L0

                # ==== A. h_{t-1}-dependent matmuls first (PE is strict
                # FIFO: nothing here may queue behind the dec_row wait) ==
                hT_in = c_WouT[:, 0:BL] if break_chain else hT[:, :]
                _mm(nc, ps_r[:, :], c_WhhT[:, 0:H], hT_in, True, False)
                _mm(nc, ps_z[:, :], c_WhhT[:, H:2 * H], hT_in,
                    True, False)
                _mm(nc, ps_hn[:, :], c_bhn[0:1, :], c_ones1[0:1, :],
                    True, False)
                _mm(nc, ps_hn[:, :], c_WhhT[:, 2 * H:H3], hT_in,
                    False, True)
                # zh = z*h and w = 1-z can also start early (need only
                # tau_z, which needs gi_z though). Emitted later.

                # ==== B. embT = relu(W_emb (x) dec_row + b_emb) =======
                _mm(nc, ps_L[:, :], c_ones1[0:1, :], c_bout[0:1, :],
                    True, False)
                dec_in = c_ident[:, :] if break_chain else diag_t[:, :]
                _mm(nc, ps_E[:, :], c_WembB[:, :], dec_in,
                    True, True)
                nc.scalar.activation(embT[:, :], ps_E[:, :],
                                     ACTF.Relu, bias=c_bemb[:, 0:1])

                # ==== C. emb-dependent matmuls ========================
                _mm(nc, ps_r[:, :], c_WihT[:, 0:H], embT[:, :], False, True)
                _mm(nc, ps_z[:, :], c_WihT[:, H:2 * H], embT[:, :],
                    False, True)
                _mm(nc, ps_in[:, :], c_WihT[:, 2 * H:H3], embT[:, :],
                    True, True)

                # ==== D. gates (sigmoid via tanh half-angle) ==========
                tau_r = wpool.tile([H, BL], F32, tag="tau_r")
                nc.scalar.activation(tau_r[:, :], ps_r[:, :], ACTF.Tanh,
                                     scale=0.5, bias=c_hbr[:, 0:1])
                tau_z = wpool.tile([H, BL], F32, tag="tau_z")
                nc.scalar.activation(tau_z[:, :], ps_z[:, :], ACTF.Tanh,
                                     scale=0.5, bias=c_hbz[:, 0:1])
                # off-chain: w = (1-z) = 0.5 - 0.5*tau_z ; zh = z*h
                w_sb = wpool.tile([H, BL], F32, tag="w_sb")
                nc.gpsimd.tensor_scalar(w_sb[:, :], tau_z[:, :],
                                        -0.5, 0.5, ALU.mult, ALU.add)
                z_sb = wpool.tile([H, BL], F32, tag="z_sb")
                nc.gpsimd.tensor_scalar(z_sb[:, :], tau_z[:, :],
                                        0.5, 0.5, ALU.mult, ALU.add)
                zh = wpool.tile([H, BL], F32, tag="zh")
                nc.gpsimd.tensor_tensor(zh[:, :], z_sb[:, :], hT_in,
                                        ALU.mult)
                # chain: tgate = (1+tau_r)*hn' ; v = tgate + i_n ; n
                tgate = wpool.tile([H, BL], F32, tag="tgate")
                nc.vector.scalar_tensor_tensor(
                    tgate[:, :], tau_r[:, :], 1.0, ps_hn[:, :],
                    ALU.add, ALU.mult)
                v_t = wpool.tile([H, BL], F32, tag="v")
                nc.vector.tensor_tensor(v_t[:, :], tgate[:, :],
                                        ps_in[:, :], ALU.add)
                n_t = wpool.tile([H, BL], F32, tag="n")
                nc.scalar.activation(n_t[:, :], v_t[:, :], ACTF.Tanh,
                                     bias=c_bihn[:, 0:1])
                # h' = n*w + zh   (2 chain hops after tanh)
                nw = wpool.tile([H, BL], F32, tag="nw")
                nc.vector.tensor_tensor(nw[:, :], n_t[:, :], w_sb[:, :],
                                        ALU.mult)
                nc.vector.tensor_tensor(hT[:, :], nw[:, :], zh[:, :],
                                        ALU.add)

                # ==== E. logits (natural layout) ======================
                _mm(nc, ps_L[:, :], hT_in[:, 0:BL] if break_chain else hT[:, 0:BL], c_WouT[:, :], False, True)

                # ==== F. topi -> dec_row (feeds step t+1) =============
                if not (last or skip_topi):
                    nc.vector.max(m8L[:, :], ps_L[:, :])
                    nc.vector.max_index(topi8[:, :], m8L[:, :], ps_L[:, :])
                    nc.vector.tensor_copy(topi_f32[:, 0:1], topi8[:, 0:1])
                    # diag(topi): off-diagonal of ident stays 0
                    nc.vector.tensor_scalar_mul(diag_t[:, :], c_ident[:, :],
                                                topi_f32[:, 0:1])

                # ==== G. masked path, lagged one step ================
                # stash this step's logits; process step t-1's masked path
                if not skip_masked:
                    if t >= 2:
                        tm = t - 1
                        lk = ps_Lprev
                        sel = wpool.tile([BL, S], F32, tag="sel")
                        prev_col = (c_iota[:, 0:1] if break_chain
                                    else chosen_st[:, tm - 1:tm])
                        nc.gpsimd.tensor_scalar(
                            sel[:, :], c_iota[:, :], prev_col,
                            NEG_BIG, ALU.is_equal, ALU.mult)
                        nc.gpsimd.tensor_tensor(maskval[:, :],
                                                maskval[:, :],
                                                sel[:, :], ALU.add)
                        masked = wpool.tile([BL, S], F32, tag="masked")
                        nc.vector.tensor_tensor(masked[:, :], lk[:, :],
                                                maskval[:, :], ALU.add)
                        nc.vector.max(m8C[:, :], masked[:, :])
                        nc.vector.max_index(chosen8[:, :], m8C[:, :],
                                            masked[:, :])
                        nc.gpsimd.tensor_copy(chosen_st[:, tm:tm + 1],
                                              chosen8[:, 0:1])
                        nc.gpsimd.tensor_scalar_mul(negm1[:, 0:1],
                                                    m8C[:, 0:1], -1.0)
                        exps = wpool.tile([BL, S], F32, tag="exps")
                        nc.scalar.activation(exps[:, :], masked[:, :],
                                             ACTF.Exp, scale=1.0,
                                             bias=negm1[:, 0:1],
                                             accum_out=s_store[:, tm:tm + 1])

            # masked path for the final step
            if n_steps >= 2 and not skip_masked:
                tm = n_steps - 1
                lk = ps_L0 if tm % 2 == 0 else ps_L1
                sel = wpool.tile([BL, S], F32, tag="sel")
                prev_col = (c_iota[:, 0:1] if break_chain
                            else chosen_st[:, tm - 1:tm])
                nc.gpsimd.tensor_scalar(
                    sel[:, :], c_iota[:, :], prev_col,
                    NEG_BIG, ALU.is_equal, ALU.mult)
                nc.gpsimd.tensor_tensor(maskval[:, :], maskval[:, :],
                                        sel[:, :], ALU.add)
                masked = wpool.tile([BL, S], F32, tag="masked")
                nc.vector.tensor_tensor(masked[:, :], lk[:, :],
                                        maskval[:, :], ALU.add)
                nc.vector.max(m8C[:, :], masked[:, :])
                nc.vector.max_index(chosen8[:, :], m8C[:, :], masked[:, :])
                nc.gpsimd.tensor_copy(chosen_st[:, tm:tm + 1],
                                      chosen8[:, 0:1])
                nc.gpsimd.tensor_scalar_mul(negm1[:, 0:1], m8C[:, 0:1],
                                            -1.0)
                exps = wpool.tile([BL, S], F32, tag="exps")
                nc.scalar.activation(exps[:, :], masked[:, :],
                                     ACTF.Exp, scale=1.0,
                                     bias=negm1[:, 0:1],
                                     accum_out=s_store[:, tm:tm + 1])

            # ---- finalize outputs -----------------------------------
            lnout = spool.tile([BL, S], F32)
            nc.scalar.activation(lnout[:, :], s_store[:, :], ACTF.Ln)
            nc.gpsimd.tensor_scalar_mul(lnout[:, :], lnout[:, :], -1.0)
            nc.sync.dma_start(d_tours[:, :], chosen_st[:, :])
            nc.sync.dma_start(d_logp[:, :], lnout[:, :])
            if rep_loop is not None:
                rep_loop.__exit__(None, None, None)

    if not nc.is_finalized():
        nc.finalize()
    return nc


def _prep_consts(W_emb, b_emb, W_ih, W_hh, b_ih, b_hh, W_out, b_out):
    """Host-side constant packing (numpy, float32)."""
    WihT = np.ascontiguousarray(W_ih.T).astype(np.float32)      # [H, 3H]
    WhhT = np.ascontiguousarray(W_hh.T).astype(np.float32)      # [H, 3H]
    WhhT[:, 2 * H:] *= 0.5                                       # hn' fold
    bias_hn = (0.5 * b_hh[2 * H:]).reshape(1, H).astype(np.float32)
    b_ihn = b_ih[2 * H:].reshape(H, 1).astype(np.float32)
    halfb_r = (0.5 * (b_ih[0:H] + b_hh[0:H])).reshape(H, 1).astype(np.float32)
    halfb_z = (0.5 * (b_ih[H:2 * H] + b_hh[H:2 * H])).reshape(H, 1).astype(
        np.float32)
    consts = {
        "WouT": np.ascontiguousarray(W_out.T).astype(np.float32),
        "WihT": WihT,
        "WhhT": WhhT,
        "WembT": W_emb.T.reshape(1, H).astype(np.float32),
        "bias_hn": bias_hn,
        "b_ihn": b_ihn,
        "halfb_r": halfb_r,
        "halfb_z": halfb_z,
        "b_out": b_out.reshape(1, S).astype(np.float32),
        "ones1": np.ones((1, H), np.float32),
        "b_emb": b_emb.reshape(H, 1).astype(np.float32),
        "iota": np.broadcast_to(np.arange(S, dtype=np.float32),
                                (BL, S)).copy(),
        "ident": np.eye(H, dtype=np.float32),
    }
    return consts


@functools.lru_cache(maxsize=1)
def _get_nc():
    return build_nc(S)


def kernel(encoder_outputs, encoder_hidden, W_emb, b_emb, W_ih, W_hh,
           b_ih, b_hh, W_out, b_out, _nc=None, _trace=False, _tmpdir=None,
           **_unused):
    encoder_hidden = np.asarray(encoder_hidden, np.float32)
    consts = _prep_consts(
        np.asarray(W_emb, np.float32), np.asarray(b_emb, np.float32),
        np.asarray(W_ih, np.float32), np.asarray(W_hh, np.float32),
        np.asarray(b_ih, np.float32), np.asarray(b_hh, np.float32),
        np.asarray(W_out, np.float32), np.asarray(b_out, np.float32))

    h0 = encoder_hidden[0]  # [B, H]
    in_maps = []
    for c in range(NCORES):
        m = dict(consts)
        m["h0T"] = np.ascontiguousarray(h0[c * BL:(c + 1) * BL].T)
        in_maps.append(m)

    nc = _nc if _nc is not None else _get_nc()
    kw = {}
    if _trace:
        kw = dict(trace=True, tmpdir=_tmpdir)
    res = run_bass_kernel_spmd(nc, in_maps, list(range(NCORES)), **kw)
    kernel.last_results = res
    tours = np.concatenate([res.results[c]["tours_o"] for c in range(NCORES)],
                           axis=0)
    logp = np.concatenate([res.results[c]["logp_o"] for c in range(NCORES)],
                          axis=0)
    tours_out = tours[:, None, :].astype(np.float32)      # [B, 1, S]
    logp_out = logp[:, :, None].astype(np.float32)        # [B, S, 1]
    return tours_out, logp_out


if __name__ == "__main__":
    import time
    t0 = time.time()
    nc = build_nc(4)
    print(f"built 4-step nc in {time.time() - t0:.1f}s")
